# revision 1
# baseline (speedup 1.0000x reference)
"""
LongConvolution (causal FFT conv) Trainium2 Bass kernel.

Problem: x (4, 8192, 1024) f32, filt (1024, 8192) f32.
  y[b, l, c] = sum_m x[b, m, c] * filt[c, l - m]   (causal, per-channel)
Reference computes this via zero-padded FFT of size N = 16384.

Strategy
--------
N = 16384 = 128 * 128 -> four-step FFT where each 128-point DFT stage is a
128x128 matmul on the tensor engine.  With n = 128*n1 + n2, k = 128*k2 + k1:

  A[k1,n2]  = sum_n1 w128^(n1 k1) u[128 n1 + n2]          (matmul vs DFT-128)
  B         = A * T,  T[k1,n2] = wN^(n2 k1)               (twiddle, DVE)
  R[k1,k2]  = sum_n2 B[k1,n2] w128^(n2 k2)                (matmul)
  R^T[k2,k1] = FFT_N(u_pad)[128 k2 + k1]  -> scrambled layout = fft().reshape!
  P = R * K  (filter spectrum K precomputed on HOST in the same layout)
  inverse: mirror image with conj twiddles; only first 64 output rows needed.

Stationary operands alternate between data (F1, I1) and constant DFT matrices
(F2, I2), which makes every stage's input layout exactly what the previous
stage produced - zero on-chip transposes.

Sharding: d_model across the 8 cores (128 channels each); each core handles
all 4 batches of its channels (filter spectrum reused across batch).

Host pre/post: x is transposed per-core to (c, b, l) so every DMA is >=512B
contiguous; output comes back as (c, b, l) and is transposed into (b, l, c).
"""

import os
import sys

import numpy as np

for p in ("/opt/trn_rl_repo",):
    if p not in sys.path:
        sys.path.insert(0, p)

os.environ.setdefault("MYCRO_LOCAL_CACHE", "1")

# ----------------------------------------------------------------------------
# configuration
# ----------------------------------------------------------------------------
B, L, D = 4, 8192, 1024
NFFT = 2 * L               # 16384 = 128 * 128
NC = 8                     # cores
CPC = D // NC              # channels per core = 128

# dtype config: "f32" (exact, slow) or fast variants
MM_DT = os.environ.get("LC_MM_DT", "f32")   # F1 matmul family: f32 | f32r
TT_DT = os.environ.get("LC_TT_DT", "f32")   # elementwise + F2/I1/I2: f32 | f16 | bf16


def _consts():
    """DFT / twiddle constant matrices (float64 -> cast at use site)."""
    j = np.arange(128)
    ang128 = 2 * np.pi * np.outer(j, j) / 128
    angN = 2 * np.pi * np.outer(j, j) / NFFT
    c = {}
    c["F_cos"] = np.cos(ang128)
    c["F_sin"] = np.sin(ang128)
    c["Tw_cos"] = np.cos(angN)
    c["Tw_sin"] = np.sin(angN)
    return c


def _build_program():
    import concourse.bacc as bacc
    import concourse.bass as bass
    import concourse.mybir as mybir
    from concourse import tile

    f32 = mybir.dt.float32
    dt_mm = {"f32": mybir.dt.float32, "f32r": mybir.dt.float32r}[MM_DT]
    dt_tt = {
        "f32": mybir.dt.float32,
        "f16": mybir.dt.float16,
        "bf16": mybir.dt.bfloat16,
    }[TT_DT]
    cast_tt = TT_DT != "f32"

    nc = bacc.Bacc(None, target_bir_lowering=False, debug=False)

    # --- DRAM I/O ---
    xw = nc.dram_tensor("xw", (CPC, B, L), dt_mm, kind="ExternalInput")
    kfre = nc.dram_tensor("kfre", (CPC, 128, 128), dt_tt, kind="ExternalInput")
    kfim = nc.dram_tensor("kfim", (CPC, 128, 128), dt_tt, kind="ExternalInput")
    f1mov_d = nc.dram_tensor("f1mov", (128, 256), dt_mm, kind="ExternalInput")
    f2re_d = nc.dram_tensor("f2re", (128, 128), dt_tt, kind="ExternalInput")
    f2im_d = nc.dram_tensor("f2im", (128, 128), dt_tt, kind="ExternalInput")
    f2sin_d = nc.dram_tensor("f2sin", (128, 128), dt_tt, kind="ExternalInput")
    fcmov_d = nc.dram_tensor("fcmov", (128, 384), dt_tt, kind="ExternalInput")
    gre_d = nc.dram_tensor("gre", (128, 64), dt_tt, kind="ExternalInput")
    gimn_d = nc.dram_tensor("gimn", (128, 64), dt_tt, kind="ExternalInput")
    t1re_d = nc.dram_tensor("t1re2", (128, 128), dt_tt, kind="ExternalInput")
    t1im_d = nc.dram_tensor("t1im2", (128, 128), dt_tt, kind="ExternalInput")
    yw = nc.dram_tensor("yw", (CPC, B, L), f32, kind="ExternalOutput")

    G = B  # all 4 batch signals of a channel processed as one group

    with tile.TileContext(nc) as tc:
        with (
            tc.tile_pool(name="const", bufs=1) as constp,
            tc.tile_pool(name="kf", bufs=4) as kfp,
            tc.tile_pool(name="m", bufs=4) as mp,
            tc.tile_pool(name="work", bufs=4) as wp,
            tc.tile_pool(name="out", bufs=4) as op,
            tc.tile_pool(name="pa", bufs=2, space="PSUM") as pap,
            tc.tile_pool(name="pr", bufs=2, space="PSUM") as prp,
            tc.tile_pool(name="pc", bufs=2, space="PSUM") as pcp,
            tc.tile_pool(name="py", bufs=2, space="PSUM") as pyp,
        ):
            # constants, DMA'd once
            f1mov = constp.tile([128, 256], dt_mm)
            f2re = constp.tile([128, 128], dt_tt)
            f2im = constp.tile([128, 128], dt_tt)
            f2sin = constp.tile([128, 128], dt_tt)
            fcmov = constp.tile([128, 384], dt_tt)
            gre = constp.tile([128, 64], dt_tt)
            gimn = constp.tile([128, 64], dt_tt)
            t1re = constp.tile([128, 128], dt_tt)
            t1im = constp.tile([128, 128], dt_tt)
            nc.sync.dma_start(f1mov[:], f1mov_d[:])
            nc.sync.dma_start(f2re[:], f2re_d[:])
            nc.sync.dma_start(f2im[:], f2im_d[:])
            nc.sync.dma_start(f2sin[:], f2sin_d[:])
            nc.sync.dma_start(fcmov[:], fcmov_d[:])
            nc.sync.dma_start(gre[:], gre_d[:])
            nc.sync.dma_start(gimn[:], gimn_d[:])
            nc.sync.dma_start(t1re[:], t1re_d[:])
            nc.sync.dma_start(t1im[:], t1im_d[:])
            t1re_b = t1re[:].rearrange("p (s n) -> p s n", s=1).broadcast_to([128, G, 128])
            t1im_b = t1im[:].rearrange("p (s n) -> p s n", s=1).broadcast_to([128, G, 128])

            for c in range(CPC):
                kre = kfp.tile([128, 128], dt_tt, tag="kre")
                kim = kfp.tile([128, 128], dt_tt, tag="kim")
                nc.sync.dma_start(kre[:], kfre[c][:])
                nc.sync.dma_start(kim[:], kfim[c][:])
                kre_b = kre[:].rearrange("p (s n) -> p s n", s=1).broadcast_to([128, G, 128])
                kim_b = kim[:].rearrange("p (s n) -> p s n", s=1).broadcast_to([128, G, 128])

                # F1: A^T = M^T @ [F_re | F_im-],  K = 64 (upper half zero)
                m4 = mp.tile([64, G, 128], dt_mm, tag="m")
                nc.sync.dma_start(
                    m4[:], xw[c].rearrange("b (a n) -> a b n", n=128)
                )
                asrc = wp.tile([128, G, 256], dt_tt, tag="asb")
                for g in range(G // 2):
                    pa = pap.tile([128, 2, 256], f32, tag="pa")
                    for i in range(2):
                        j = 2 * g + i
                        nc.tensor.matmul(
                            pa[:, i, :], m4[:, j, :], f1mov[0:64, :],
                            start=True, stop=True,
                        )
                    nc.scalar.copy(
                        out=asrc[:, 2 * g : 2 * g + 2, :], in_=pa[:]
                    )

                # T1 twiddle: B = A * T1
                a_re = asrc[:, :, 0:128]
                a_im = asrc[:, :, 128:256]
                u1 = wp.tile([128, G, 128], dt_tt, tag="u1")
                u2 = wp.tile([128, G, 128], dt_tt, tag="u2")
                u3 = wp.tile([128, G, 128], dt_tt, tag="u3")
                u4 = wp.tile([128, G, 128], dt_tt, tag="u4")
                b_t = wp.tile([128, G, 256], dt_tt, tag="b")
                nc.vector.tensor_mul(u1[:], a_re, t1re_b)
                nc.vector.tensor_mul(u2[:], a_im, t1im_b)
                nc.vector.tensor_sub(b_t[:, :, 0:128], u1[:], u2[:])
                nc.vector.tensor_mul(u3[:], a_re, t1im_b)
                nc.vector.tensor_mul(u4[:], a_im, t1re_b)
                nc.vector.tensor_add(b_t[:, :, 128:256], u3[:], u4[:])

                # F2: R^T = F- @ B^T  [k2, k1]; sign of the sin-part lives in
                # the constants (f2sin / f2im), so no negated-B tile is needed
                rsrc = wp.tile([128, G, 256], dt_tt, tag="rsb")
                for g in range(G // 2):  # one psum bank per 2 signals
                    sl = slice(2 * g, 2 * g + 2)
                    b_re = b_t[:, sl, 0:128]
                    b_im = b_t[:, sl, 128:256]
                    pr = prp.tile([128, 2, 256], f32, tag="pr")
                    nc.tensor.matmul(
                        pr[:, :, 0:128], f2re[:], b_re, start=True, stop=False
                    )
                    nc.tensor.matmul(
                        pr[:, :, 0:128], f2sin[:], b_im, start=False, stop=True
                    )
                    nc.tensor.matmul(
                        pr[:, :, 128:256], f2re[:], b_im, start=True, stop=False
                    )
                    nc.tensor.matmul(
                        pr[:, :, 128:256], f2im[:], b_re, start=False, stop=True
                    )
                    nc.scalar.copy(out=rsrc[:, sl, :], in_=pr[:])

                # pointwise with filter spectrum: P = R * K
                r_re = rsrc[:, :, 0:128]
                r_im = rsrc[:, :, 128:256]
                v1 = wp.tile([128, G, 128], dt_tt, tag="u1")
                v2 = wp.tile([128, G, 128], dt_tt, tag="u2")
                v3 = wp.tile([128, G, 128], dt_tt, tag="u3")
                v4 = wp.tile([128, G, 128], dt_tt, tag="u4")
                p_re = wp.tile([128, G, 128], dt_tt, tag="pre")
                p_im = wp.tile([128, G, 128], dt_tt, tag="pim")
                nc.vector.tensor_mul(v1[:], r_re, kre_b)
                nc.vector.tensor_mul(v2[:], r_im, kim_b)
                nc.vector.tensor_sub(p_re[:], v1[:], v2[:])
                nc.vector.tensor_mul(v3[:], r_re, kim_b)
                nc.vector.tensor_mul(v4[:], r_im, kre_b)
                nc.vector.tensor_add(p_im[:], v3[:], v4[:])

                # I1: C = P @ F+   [k1, n2]  (data-stationary)
                csrc = wp.tile([128, G, 256], dt_tt, tag="csb")
                for g in range(G // 2):
                    pc = pcp.tile([128, 2, 256], f32, tag="pc")
                    for i in range(2):
                        j = 2 * g + i
                        nc.tensor.matmul(
                            pc[:, i, :], p_re[:, j, :], fcmov[:, 128:384],
                            start=True, stop=False,
                        )
                        nc.tensor.matmul(
                            pc[:, i, :], p_im[:, j, :], fcmov[:, 0:256],
                            start=False, stop=True,
                        )
                    nc.scalar.copy(
                        out=csrc[:, 2 * g : 2 * g + 2, :], in_=pc[:]
                    )

                # T2 twiddle: C' = C * conj(T1)
                c_re = csrc[:, :, 0:128]
                c_im = csrc[:, :, 128:256]
                w1 = wp.tile([128, G, 128], dt_tt, tag="u1")
                w2 = wp.tile([128, G, 128], dt_tt, tag="u2")
                w3 = wp.tile([128, G, 128], dt_tt, tag="u3")
                w4 = wp.tile([128, G, 128], dt_tt, tag="u4")
                cp_re = wp.tile([128, G, 128], dt_tt, tag="cpre")
                cp_im = wp.tile([128, G, 128], dt_tt, tag="cpim")
                nc.vector.tensor_mul(w1[:], c_re, t1re_b)
                nc.vector.tensor_mul(w2[:], c_im, t1im_b)
                nc.vector.tensor_add(cp_re[:], w1[:], w2[:])
                nc.vector.tensor_mul(w3[:], c_re, t1im_b)
                nc.vector.tensor_mul(w4[:], c_im, t1re_b)
                nc.vector.tensor_sub(cp_im[:], w4[:], w3[:])

                # I2: y = Re(F+ @ C'), first 64 rows; 1/N folded into K
                ysb = op.tile([64, G, 128], f32, tag="ysb")
                for g in range(G // 2):
                    sl = slice(2 * g, 2 * g + 2)
                    py = pyp.tile([64, 2, 128], f32, tag="py")
                    nc.tensor.matmul(
                        py[:], gre[:], cp_re[:, sl, :], start=True, stop=False
                    )
                    nc.tensor.matmul(
                        py[:], gimn[:], cp_im[:, sl, :], start=False, stop=True
                    )
                    nc.scalar.copy(out=ysb[:, sl, :], in_=py[:])
                nc.sync.dma_start(
                    yw[c].rearrange("b (a n) -> a b n", n=128), ysb[:]
                )

    nc.compile()
    return nc


def _host_arrays():
    cst = _consts()
    F_cos, F_sin = cst["F_cos"], cst["F_sin"]
    Tw_cos, Tw_sin = cst["Tw_cos"], cst["Tw_sin"]

    np_tt = {"f32": np.float32, "f16": np.float16, "bf16": None}[TT_DT]
    if np_tt is None:
        import ml_dtypes

        np_tt = ml_dtypes.bfloat16
    np_mm = np.float32

    arrs = {}
    arrs["f1mov"] = np.concatenate([F_cos, -F_sin], axis=1).astype(np_mm)
    arrs["f2re"] = F_cos.astype(np_tt)
    arrs["f2im"] = (-F_sin).astype(np_tt)
    arrs["f2sin"] = F_sin.astype(np_tt)
    # fcmov = [F+_im_neg | F+_re | F+_im] = [-sin | cos | sin]
    arrs["fcmov"] = np.concatenate([-F_sin, F_cos, F_sin], axis=1).astype(np_tt)
    # 1/NFFT normalization lives in the host-side filter spectrum (keeps
    # every on-chip intermediate within fp16 range)
    arrs["gre"] = F_cos[:, :64].astype(np_tt)
    arrs["gimn"] = (-F_sin[:, :64]).astype(np_tt)
    arrs["t1re2"] = Tw_cos.astype(np_tt)
    arrs["t1im2"] = (-Tw_sin).astype(np_tt)
    return arrs, np_tt


def kernel(x: np.ndarray, filt: np.ndarray) -> np.ndarray:
    from concourse.bass_utils import run_bass_kernel_spmd

    assert x.shape == (B, L, D) and filt.shape == (D, L)
    x = np.ascontiguousarray(x, dtype=np.float32)
    filt = np.ascontiguousarray(filt, dtype=np.float32)

    consts, np_tt = _host_arrays()

    # filter spectrum: FFT of zero-padded filter; reshape(128,128) IS the
    # scrambled [k2,k1] layout produced by the on-device four-step forward.
    kpad = np.zeros((D, NFFT), np.float64)
    kpad[:, :L] = filt
    Kf = (np.fft.fft(kpad, axis=1) / NFFT).reshape(D, 128, 128)

    in_maps = []
    for ci in range(NC):
        sl = slice(ci * CPC, (ci + 1) * CPC)
        m = dict(consts)
        m["xw"] = np.ascontiguousarray(x[:, :, sl].transpose(2, 0, 1))
        m["kfre"] = np.ascontiguousarray(Kf[sl].real.astype(np_tt))
        m["kfim"] = np.ascontiguousarray(Kf[sl].imag.astype(np_tt))
        in_maps.append(m)

    nc = _build_program()
    res = run_bass_kernel_spmd(nc, in_maps, core_ids=list(range(NC)))

    y = np.empty((B, L, D), np.float32)
    for ci in range(NC):
        sl = slice(ci * CPC, (ci + 1) * CPC)
        y[:, :, sl] = res.results[ci]["yw"].transpose(1, 2, 0)
    return y


def run_profiled(inputs):
    """Build + run with NTFF tracing; returns BassKernelResults (test-only)."""
    from concourse.bass_utils import run_bass_kernel_spmd

    x = np.ascontiguousarray(inputs["x"], dtype=np.float32)
    filt = np.ascontiguousarray(inputs["filt"], dtype=np.float32)
    consts, np_tt = _host_arrays()
    kpad = np.zeros((D, NFFT), np.float64)
    kpad[:, :L] = filt
    Kf = (np.fft.fft(kpad, axis=1) / NFFT).reshape(D, 128, 128)
    in_maps = []
    for ci in range(NC):
        sl = slice(ci * CPC, (ci + 1) * CPC)
        m = dict(consts)
        m["xw"] = np.ascontiguousarray(x[:, :, sl].transpose(2, 0, 1))
        m["kfre"] = np.ascontiguousarray(Kf[sl].real.astype(np_tt))
        m["kfim"] = np.ascontiguousarray(Kf[sl].imag.astype(np_tt))
        in_maps.append(m)
    nc = _build_program()
    return run_bass_kernel_spmd(
        nc, in_maps, core_ids=list(range(NC)), trace=True
    )


if __name__ == "__main__":
    rng = np.random.default_rng(0)
    x = rng.standard_normal((B, L, D)).astype(np.float32)
    filt = rng.standard_normal((D, L)).astype(np.float32)
    y = kernel(x, filt)
    print("y", y.shape, y.dtype, float(np.abs(y).max()))



# revision 18
# speedup vs baseline: 6.2863x; 6.2863x over previous
"""
LongConvolution (causal FFT conv) Trainium2 Bass kernel, v2.

Problem: x (4, 8192, 1024) f32, filt (1024, 8192) f32.
  y[b, l, c] = sum_m x[b, m, c] * filt[c, l - m]   (causal, per-channel)

Strategy
--------
N = 16384 = 128*128 four-step FFT where each 128-point DFT stage is a
matmul on the tensor engine (fp16 in, f32 PSUM accumulate).

Complex batch packing: convolution is linear in the signal, so batches
(b0,b1) pack into ONE complex signal z = b0 + i*b1. The per-channel
filter multiply commutes with the packing (K*(U0 + i U1) = U0*K + i U1*K),
so y0 = Re(result), y1 = Im(result) with no unpack step. This halves the
F2/I1 matmul volume and all elementwise twiddle/pointwise work vs
processing 4 real signals.

Per complex signal (layout notes, partition dim first):
  m   [128, 128]   K-stacked [zre(n1=0:64); zim] ; l = 128*n1 + n2
  F1  A[n2, k1]    = m^T @ f1k            (1 matmul, N=256: [Are|Aim])
  T1  B = A * (TC - i*TS)                 (DVE quad-mul + 2 subs)
  F2  R[k2, k1]    = C@Bre + S@Bim | C@Bim - S@Bre   (const-stationary)
  PW  P = R * K                           (DVE quad-mul, gpsimd combines)
  I1  Q[k1, n2]    = P^T @ (C + iS)       (data-stationary, 2 matmuls)
  T2  W quad only: C' = Q * (TC + i*TS)   (DVE quad-mul; combine FOLDED
                                           into I2's PSUM accumulation)
  I2  y[n1, n2]    = Re/Im of G @ C', n1 < 64  (8 matmuls of N=256/pair)

Sharding: d_model across 8 cores (128 channels each); channels processed
in groups of GC=2 to amortize DVE instruction overhead.

Engines: PE ~4.6k cyc/ch @2.4GHz, DVE ~2.1us/ch (fp16 2x mode),
ACT does all PSUM->SBUF casts, gpsimd does the PW combines.

Host pre/post: x is cast to fp16 and transposed per-core to (c, b, l);
filter spectrum precomputed on host (fp16, scrambled [k2,k1] layout,
1/N folded in); output comes back fp16 (c, b, l) -> f32 (b, l, c).
"""

import os
import sys

import numpy as np

for p in ("/opt/trn_rl_repo",):
    if p not in sys.path:
        sys.path.insert(0, p)

os.environ.setdefault("MYCRO_LOCAL_CACHE", "1")

# ----------------------------------------------------------------------------
# configuration
# ----------------------------------------------------------------------------
B, L, D = 4, 8192, 1024
NFFT = 2 * L               # 16384 = 128 * 128
NC = 8                     # cores
CPC = D // NC              # channels per core = 128
GC = 2                     # channels per group (DVE batching)
NG = CPC // GC             # 64 groups
U = GC * 2                 # signal slots per group (2 complex per channel)

# tuning knobs (build-time)
PW_ON_GPSIMD = os.environ.get("LC_PW_GPSIMD", "1") == "1"
FOLD_T1 = os.environ.get("LC_FOLD_T1", "0") == "1"   # fold T1 combine into F2
FOLD_T2 = os.environ.get("LC_FOLD_T2", "1") == "1"   # fold T2 combine into I2
DEBUG_DUMP = os.environ.get("LC_DEBUG", "0") == "1"  # dump group-0 intermediates


def _consts():
    j = np.arange(128)
    ang128 = 2 * np.pi * np.outer(j, j) / 128
    angN = 2 * np.pi * np.outer(j, j) / NFFT
    C, S = np.cos(ang128), np.sin(ang128)
    TC, TS = np.cos(angN), np.sin(angN)
    C64, S64 = C[:64], S[:64]

    arrs = {}
    # F1 rhs: rows 0:64 (zre): [cos | -sin]; rows 64:128 (zim): [sin | cos]
    arrs["f1k"] = np.block([[C64, -S64], [S64, C64]])
    # T1 quad consts: q = [Are|Aim|Are|Aim] * [TC|-TS|-TS|-TC]
    #   Bre = q0 - q1 ; Bim = q2 - q3
    arrs["t1q"] = np.concatenate([TC, -TS, -TS, -TC], axis=1)
    # T2 quad consts: q = [Qre|Qim|Qre|Qim] * [TC|TS|TS|-TC]
    #   C're = q0 - q1 ; C'im = q2 - q3
    arrs["t2q"] = np.concatenate([TC, TS, TS, -TC], axis=1)
    # F2 stationaries
    arrs["f2c"] = C
    arrs["f2s"] = S
    arrs["f2sn"] = -S
    arrs["f2cn"] = -C
    # I1 rhs: [-S | C | S]; rhs1 = [C|S] (cols 128:384), rhs2 = [-S|C] (0:256)
    arrs["i1m"] = np.concatenate([-S, C, S], axis=1)
    # I2 stationaries [k1, n1], n1 < 64
    arrs["gc"] = C[:, :64]
    arrs["gcn"] = -C[:, :64]
    arrs["gs"] = S[:, :64]
    arrs["gsn"] = -S[:, :64]
    return arrs


def _build_program():
    import concourse.bacc as bacc
    import concourse.mybir as mybir
    from concourse import tile

    f16 = mybir.dt.float16
    f32 = mybir.dt.float32

    nc = bacc.Bacc(None, target_bir_lowering=False, debug=False)

    # --- DRAM I/O ---
    xw = nc.dram_tensor("xw", (CPC, B, L), f16, kind="ExternalInput")
    kfq = nc.dram_tensor("kfq", (NG, 128, GC, 384), f16, kind="ExternalInput")
    f1k_d = nc.dram_tensor("f1k", (128, 256), f16, kind="ExternalInput")
    t1q_d = nc.dram_tensor("t1q", (128, 512), f16, kind="ExternalInput")
    t2q_d = nc.dram_tensor("t2q", (128, 512), f16, kind="ExternalInput")
    f2c_d = nc.dram_tensor("f2c", (128, 128), f16, kind="ExternalInput")
    f2s_d = nc.dram_tensor("f2s", (128, 128), f16, kind="ExternalInput")
    f2sn_d = nc.dram_tensor("f2sn", (128, 128), f16, kind="ExternalInput")
    f2cn_d = nc.dram_tensor("f2cn", (128, 128), f16, kind="ExternalInput")
    i1m_d = nc.dram_tensor("i1m", (128, 384), f16, kind="ExternalInput")
    gc_d = nc.dram_tensor("gc", (128, 64), f16, kind="ExternalInput")
    gcn_d = nc.dram_tensor("gcn", (128, 64), f16, kind="ExternalInput")
    gs_d = nc.dram_tensor("gs", (128, 64), f16, kind="ExternalInput")
    gsn_d = nc.dram_tensor("gsn", (128, 64), f16, kind="ExternalInput")
    yw = nc.dram_tensor("yw", (CPC, B, L), f16, kind="ExternalOutput")
    dbg = {}
    if DEBUG_DUMP:
        for nm, shp in (
            ("dA", (128, U, 256)), ("dB", (128, U, 256)), ("dR", (128, U, 256)),
            ("dP", (128, U, 256)), ("dQ", (128, U, 256)), ("dW", (128, U, 512)),
        ):
            dbg[nm] = nc.dram_tensor(nm, shp, f16, kind="ExternalOutput")

    with tile.TileContext(nc) as tc:
        with (
            tc.tile_pool(name="const", bufs=1) as constp,
            tc.tile_pool(name="kf", bufs=3) as kfp,
            tc.tile_pool(name="m", bufs=3) as mp,
            tc.tile_pool(name="work", bufs=3) as wp,
            tc.tile_pool(name="out", bufs=3) as op,
            tc.tile_pool(name="pa", bufs=2, space="PSUM") as pap,
            tc.tile_pool(name="pr", bufs=2, space="PSUM") as prp,
            tc.tile_pool(name="pc", bufs=2, space="PSUM") as pcp,
            tc.tile_pool(name="py", bufs=2, space="PSUM") as pyp,
        ):
            # constants, DMA'd once
            f1k = constp.tile([128, 256], f16)
            t1q = constp.tile([128, 512], f16)
            t2q = constp.tile([128, 512], f16)
            f2c = constp.tile([128, 128], f16)
            f2s = constp.tile([128, 128], f16)
            f2sn = constp.tile([128, 128], f16)
            f2cn = constp.tile([128, 128], f16)
            i1m = constp.tile([128, 384], f16)
            gc = constp.tile([128, 64], f16)
            gcn = constp.tile([128, 64], f16)
            gs = constp.tile([128, 64], f16)
            gsn = constp.tile([128, 64], f16)
            for t, d in (
                (f1k, f1k_d), (t1q, t1q_d), (t2q, t2q_d), (f2c, f2c_d),
                (f2s, f2s_d), (f2sn, f2sn_d), (f2cn, f2cn_d), (i1m, i1m_d),
                (gc, gc_d), (gcn, gcn_d), (gs, gs_d), (gsn, gsn_d),
            ):
                nc.sync.dma_start(t[:], d[:])

            def bcast(ap, n):
                # [128, n_cols] -> [128, U, n_cols] free-dim broadcast
                return (
                    ap.rearrange("p (u n) -> p u n", u=1).broadcast_to([128, U, n])
                )

            for g in range(NG):
                kf = kfp.tile([128, GC, 384], f16, tag="kf")
                nc.sync.dma_start(kf[:], kfq[g][:])

                m = mp.tile([128, U, 128], f16, tag="m")
                for j in range(GC):
                    c = g * GC + j
                    for s in range(2):
                        nc.sync.dma_start(
                            m[:, 2 * j + s, :],
                            xw[c][2 * s : 2 * s + 2].rearrange(
                                "b (a n) -> (b a) n", n=128
                            ),
                        )

                # ---- F1: A[n2, k1] = m^T @ f1k, per signal slot ----
                A_sb = wp.tile([128, U, 256], f16, tag="A")
                for j in range(GC):
                    pa = pap.tile([128, 2, 256], f32, tag="pa")
                    for s in range(2):
                        nc.tensor.matmul(
                            pa[:, s, :], m[:, 2 * j + s, :], f1k[:],
                            start=True, stop=True,
                        )
                    nc.scalar.copy(out=A_sb[:, 2 * j : 2 * j + 2, :], in_=pa[:])
                if DEBUG_DUMP and g == 0:
                    nc.sync.dma_start(dbg["dA"][:], A_sb[:])

                # ---- T1 quad: tq = [A|A] * [TC|-TS|-TS|-TC] ----
                tq = wp.tile([128, U, 512], f16, tag="tq")
                nc.vector.tensor_mul(
                    tq[:, :, 0:256], A_sb[:], bcast(t1q[:, 0:256], 256)
                )
                nc.vector.tensor_mul(
                    tq[:, :, 256:512], A_sb[:], bcast(t1q[:, 256:512], 256)
                )

                if not FOLD_T1:
                    B_sb = wp.tile([128, U, 256], f16, tag="B")
                    nc.vector.tensor_sub(
                        B_sb[:, :, 0:128], tq[:, :, 0:128], tq[:, :, 128:256]
                    )
                    nc.vector.tensor_sub(
                        B_sb[:, :, 128:256], tq[:, :, 256:384], tq[:, :, 384:512]
                    )
                    if DEBUG_DUMP and g == 0:
                        nc.sync.dma_start(dbg["dB"][:], B_sb[:])

                # ---- F2: R[k2, k1] (const-stationary, grouped) ----
                # re = C@Bre + S@Bim ; im = C@Bim - S@Bre
                R_sb = wp.tile([128, U, 256], f16, tag="R")
                prs = [prp.tile([128, 2, 256], f32, tag="pr", name=f"pr{_j}") for _j in range(GC)]
                # NOTE: start_tensor_calc marks the bank's ENTIRE zero region
                # pending-zero, so each PSUM bank gets exactly one start (its
                # first matmul) and one stop (its last); first touch of every
                # byte range overwrites via the bank-wide pending-zero.
                if FOLD_T1:
                    # Bre = q0 - q1, Bim = q2 - q3 (q = tq slots of 128)
                    def q(j, k):
                        return tq[:, 2 * j : 2 * j + 2, 128 * k : 128 * (k + 1)]

                    # re: C@q0 + Cn@q1 + S@q2 + Sn@q3
                    # im: C@q2 + Cn@q3 + S@q1 + Sn@q0
                    for j in range(GC):
                        nc.tensor.matmul(prs[j][:, :, 0:128], f2c[:], q(j, 0), start=True, stop=False)
                    for j in range(GC):
                        nc.tensor.matmul(prs[j][:, :, 128:256], f2c[:], q(j, 2), start=False, stop=False)
                    for j in range(GC):
                        nc.tensor.matmul(prs[j][:, :, 0:128], f2cn[:], q(j, 1), start=False, stop=False)
                    for j in range(GC):
                        nc.tensor.matmul(prs[j][:, :, 128:256], f2cn[:], q(j, 3), start=False, stop=False)
                    for j in range(GC):
                        nc.tensor.matmul(prs[j][:, :, 0:128], f2s[:], q(j, 2), start=False, stop=False)
                    for j in range(GC):
                        nc.tensor.matmul(prs[j][:, :, 128:256], f2s[:], q(j, 1), start=False, stop=False)
                    for j in range(GC):
                        nc.tensor.matmul(prs[j][:, :, 0:128], f2sn[:], q(j, 3), start=False, stop=False)
                    for j in range(GC):
                        nc.tensor.matmul(prs[j][:, :, 128:256], f2sn[:], q(j, 0), start=False, stop=True)
                else:
                    def bre(j):
                        return B_sb[:, 2 * j : 2 * j + 2, 0:128]

                    def bim(j):
                        return B_sb[:, 2 * j : 2 * j + 2, 128:256]

                    for j in range(GC):
                        nc.tensor.matmul(prs[j][:, :, 0:128], f2c[:], bre(j), start=True, stop=False)
                    for j in range(GC):
                        nc.tensor.matmul(prs[j][:, :, 128:256], f2c[:], bim(j), start=False, stop=False)
                    for j in range(GC):
                        nc.tensor.matmul(prs[j][:, :, 0:128], f2s[:], bim(j), start=False, stop=False)
                    for j in range(GC):
                        nc.tensor.matmul(prs[j][:, :, 128:256], f2sn[:], bre(j), start=False, stop=True)
                for j in range(GC):
                    nc.scalar.copy(
                        out=R_sb[:, 2 * j : 2 * j + 2, :], in_=prs[j][:]
                    )
                if DEBUG_DUMP and g == 0:
                    nc.sync.dma_start(dbg["dR"][:], R_sb[:])

                # ---- PW quads: pq = [R|R] * [Kre|Kim] / [Kim|Kre] ----
                pq = wp.tile([128, U, 512], f16, tag="pq")
                kf_a = (
                    kf[:, :, 0:256]
                    .rearrange("p c (s n) -> p c s n", s=1)
                    .broadcast_to([128, GC, 2, 256])
                )
                kf_b = (
                    kf[:, :, 128:384]
                    .rearrange("p c (s n) -> p c s n", s=1)
                    .broadcast_to([128, GC, 2, 256])
                )
                pq_v = pq[:].rearrange("p (c s) n -> p c s n", c=GC)
                R_v = R_sb[:].rearrange("p (c s) n -> p c s n", c=GC)
                nc.vector.tensor_mul(pq_v[:, :, :, 0:256], R_v, kf_a)
                nc.vector.tensor_mul(pq_v[:, :, :, 256:512], R_v, kf_b)

                # ---- PW combines: Pre = q0 - q1 ; Pim = q2 + q3 ----
                P_sb = wp.tile([128, U, 256], f16, tag="P")
                eng = nc.gpsimd if PW_ON_GPSIMD else nc.vector
                eng.tensor_sub(
                    P_sb[:, :, 0:128], pq[:, :, 0:128], pq[:, :, 128:256]
                )
                eng.tensor_add(
                    P_sb[:, :, 128:256], pq[:, :, 256:384], pq[:, :, 384:512]
                )
                if DEBUG_DUMP and g == 0:
                    nc.sync.dma_start(dbg["dP"][:], P_sb[:])

                # ---- I1: Q[k1, n2] = P^T @ (C + iS), data-stationary ----
                Q_sb = wp.tile([128, U, 256], f16, tag="Q")
                for j in range(GC):
                    pc = pcp.tile([128, 2, 256], f32, tag="pc")
                    for s in range(2):
                        u = 2 * j + s
                        nc.tensor.matmul(
                            pc[:, s, :], P_sb[:, u, 0:128], i1m[:, 128:384],
                            start=True, stop=False,
                        )
                        nc.tensor.matmul(
                            pc[:, s, :], P_sb[:, u, 128:256], i1m[:, 0:256],
                            start=False, stop=True,
                        )
                    nc.scalar.copy(out=Q_sb[:, 2 * j : 2 * j + 2, :], in_=pc[:])
                if DEBUG_DUMP and g == 0:
                    nc.sync.dma_start(dbg["dQ"][:], Q_sb[:])

                # ---- T2 quad: wq = [Q|Q] * [TC|TS|TS|-TC] ----
                wq = wp.tile([128, U, 512], f16, tag="wq")
                nc.vector.tensor_mul(
                    wq[:, :, 0:256], Q_sb[:], bcast(t2q[:, 0:256], 256)
                )
                nc.vector.tensor_mul(
                    wq[:, :, 256:512], Q_sb[:], bcast(t2q[:, 256:512], 256)
                )
                if DEBUG_DUMP and g == 0:
                    nc.sync.dma_start(dbg["dW"][:], wq[:])

                # ---- I2 (T2 combine folded): ----
                # C're = w0 - w1, C'im = w2 - w3
                # y_re = gc@C're - gs@C'im ; y_im = gc@C'im + gs@C're
                ysb = op.tile([64, U, 256], f16, tag="ysb")
                pys = [pyp.tile([64, 2, 256], f32, tag="py", name=f"py{_j}") for _j in range(GC)]
                if FOLD_T2:
                    def w(j, k):
                        return wq[:, 2 * j : 2 * j + 2, 128 * k : 128 * (k + 1)]

                    # re: gc@w0 + gcn@w1 + gsn@w2 + gs@w3
                    # im: gc@w2 + gcn@w3 + gs@w0 + gsn@w1
                    for j in range(GC):
                        nc.tensor.matmul(pys[j][:, :, 0:128], gc[:], w(j, 0), start=True, stop=False)
                    for j in range(GC):
                        nc.tensor.matmul(pys[j][:, :, 128:256], gc[:], w(j, 2), start=False, stop=False)
                    for j in range(GC):
                        nc.tensor.matmul(pys[j][:, :, 0:128], gcn[:], w(j, 1), start=False, stop=False)
                    for j in range(GC):
                        nc.tensor.matmul(pys[j][:, :, 128:256], gcn[:], w(j, 3), start=False, stop=False)
                    for j in range(GC):
                        nc.tensor.matmul(pys[j][:, :, 0:128], gsn[:], w(j, 2), start=False, stop=False)
                    for j in range(GC):
                        nc.tensor.matmul(pys[j][:, :, 128:256], gsn[:], w(j, 1), start=False, stop=False)
                    for j in range(GC):
                        nc.tensor.matmul(pys[j][:, :, 0:128], gs[:], w(j, 3), start=False, stop=False)
                    for j in range(GC):
                        nc.tensor.matmul(pys[j][:, :, 128:256], gs[:], w(j, 0), start=False, stop=True)
                else:
                    # need explicit C' tile
                    C_sb = wp.tile([128, U, 256], f16, tag="C")
                    nc.vector.tensor_sub(
                        C_sb[:, :, 0:128], wq[:, :, 0:128], wq[:, :, 128:256]
                    )
                    nc.vector.tensor_sub(
                        C_sb[:, :, 128:256], wq[:, :, 256:384], wq[:, :, 384:512]
                    )

                    def cre(j):
                        return C_sb[:, 2 * j : 2 * j + 2, 0:128]

                    def cim(j):
                        return C_sb[:, 2 * j : 2 * j + 2, 128:256]

                    for j in range(GC):
                        nc.tensor.matmul(pys[j][:, :, 0:128], gc[:], cre(j), start=True, stop=False)
                    for j in range(GC):
                        nc.tensor.matmul(pys[j][:, :, 128:256], gc[:], cim(j), start=False, stop=False)
                    for j in range(GC):
                        nc.tensor.matmul(pys[j][:, :, 0:128], gsn[:], cim(j), start=False, stop=False)
                    for j in range(GC):
                        nc.tensor.matmul(pys[j][:, :, 128:256], gs[:], cre(j), start=False, stop=True)

                for j in range(GC):
                    nc.scalar.copy(
                        out=ysb[:, 2 * j : 2 * j + 2, :], in_=pys[j][:]
                    )

                # ---- store: y_re -> batch 2s, y_im -> batch 2s+1 ----
                for j in range(GC):
                    c = g * GC + j
                    for s in range(2):
                        nc.sync.dma_start(
                            yw[c][2 * s : 2 * s + 2].rearrange(
                                "b (a n) -> a b n", n=128
                            ),
                            ysb[:, 2 * j + s, :].rearrange(
                                "p (b n) -> p b n", n=128
                            ),
                        )

    nc.compile()
    return nc


def _host_arrays():
    cst = _consts()
    return {k: v.astype(np.float16) for k, v in cst.items()}


def _prep_inputs(x, filt):
    consts = _host_arrays()

    # filter spectrum: FFT of zero-padded filter; reshape(128,128) IS the
    # scrambled [k2, k1] layout of the on-device four-step forward.
    kpad = np.zeros((D, NFFT), np.float64)
    kpad[:, :L] = filt
    Kf = (np.fft.fft(kpad, axis=1) / NFFT).reshape(D, 128, 128)
    Kre = Kf.real.astype(np.float16)
    Kim = Kf.imag.astype(np.float16)
    # per-channel [128, 384] = [Kre | Kim | Kre]
    kq = np.concatenate([Kre, Kim, Kre], axis=2)  # (D, 128, 384)

    x16 = x.astype(np.float16)
    in_maps = []
    for ci in range(NC):
        sl = slice(ci * CPC, (ci + 1) * CPC)
        m = dict(consts)
        m["xw"] = np.ascontiguousarray(x16[:, :, sl].transpose(2, 0, 1))
        # kfq[g, p, j, :] = kq[core_base + 2g + j, p, :]
        m["kfq"] = np.ascontiguousarray(
            kq[sl].reshape(NG, GC, 128, 384).transpose(0, 2, 1, 3)
        )
        in_maps.append(m)
    return in_maps


def kernel(x: np.ndarray, filt: np.ndarray) -> np.ndarray:
    from concourse.bass_utils import run_bass_kernel_spmd

    assert x.shape == (B, L, D) and filt.shape == (D, L)
    x = np.ascontiguousarray(x, dtype=np.float32)
    filt = np.ascontiguousarray(filt, dtype=np.float32)

    in_maps = _prep_inputs(x, filt)
    nc = _build_program()
    res = run_bass_kernel_spmd(nc, in_maps, core_ids=list(range(NC)))

    y = np.empty((B, L, D), np.float32)
    for ci in range(NC):
        sl = slice(ci * CPC, (ci + 1) * CPC)
        y[:, :, sl] = res.results[ci]["yw"].astype(np.float32).transpose(1, 2, 0)
    return y


def run_profiled(inputs):
    """Build + run with NTFF tracing; returns BassKernelResults (test-only)."""
    from concourse.bass_utils import run_bass_kernel_spmd

    x = np.ascontiguousarray(inputs["x"], dtype=np.float32)
    filt = np.ascontiguousarray(inputs["filt"], dtype=np.float32)
    in_maps = _prep_inputs(x, filt)
    nc = _build_program()
    return run_bass_kernel_spmd(
        nc, in_maps, core_ids=list(range(NC)), trace=True
    )


if __name__ == "__main__":
    rng = np.random.default_rng(0)
    x = rng.standard_normal((B, L, D)).astype(np.float32)
    filt = rng.standard_normal((D, L)).astype(np.float32)
    y = kernel(x, filt)
    print("y", y.shape, y.dtype, float(np.abs(y).max()))


# revision 21
# speedup vs baseline: 8.7843x; 1.3974x over previous
"""
LongConvolution (causal FFT conv) Trainium2 Bass kernel, v2.

Problem: x (4, 8192, 1024) f32, filt (1024, 8192) f32.
  y[b, l, c] = sum_m x[b, m, c] * filt[c, l - m]   (causal, per-channel)

Strategy
--------
N = 16384 = 128*128 four-step FFT where each 128-point DFT stage is a
matmul on the tensor engine (fp16 in, f32 PSUM accumulate).

Complex batch packing: convolution is linear in the signal, so batches
(b0,b1) pack into ONE complex signal z = b0 + i*b1. The per-channel
filter multiply commutes with the packing (K*(U0 + i U1) = U0*K + i U1*K),
so y0 = Re(result), y1 = Im(result) with no unpack step. This halves the
F2/I1 matmul volume and all elementwise twiddle/pointwise work vs
processing 4 real signals.

Per complex signal (layout notes, partition dim first):
  m   [128, 128]   K-stacked [zre(n1=0:64); zim] ; l = 128*n1 + n2
  F1  A[n2, k1]    = m^T @ f1k            (1 matmul, N=256: [Are|Aim])
  T1  B = A * (TC - i*TS)                 (DVE quad-mul + 2 subs)
  F2  R[k2, k1]    = C@Bre + S@Bim | C@Bim - S@Bre   (const-stationary)
  PW  P = R * K                           (DVE quad-mul, gpsimd combines)
  I1  Q[k1, n2]    = P^T @ (C + iS)       (data-stationary, 2 matmuls)
  T2  W quad only: C' = Q * (TC + i*TS)   (DVE quad-mul; combine FOLDED
                                           into I2's PSUM accumulation)
  I2  y[n1, n2]    = Re/Im of G @ C', n1 < 64  (8 matmuls of N=256/pair)

Sharding: d_model across 8 cores (128 channels each); channels processed
in groups of GC=2 to amortize DVE instruction overhead.

Engines: PE ~4.6k cyc/ch @2.4GHz, DVE ~2.1us/ch (fp16 2x mode),
ACT does all PSUM->SBUF casts, gpsimd does the PW combines.

Host pre/post: x is cast to fp16 and transposed per-core to (c, b, l);
filter spectrum precomputed on host (fp16, scrambled [k2,k1] layout,
1/N folded in); output comes back fp16 (c, b, l) -> f32 (b, l, c).
"""

import os
import sys

import numpy as np

for p in ("/opt/trn_rl_repo",):
    if p not in sys.path:
        sys.path.insert(0, p)

os.environ.setdefault("MYCRO_LOCAL_CACHE", "1")

# ----------------------------------------------------------------------------
# configuration
# ----------------------------------------------------------------------------
B, L, D = 4, 8192, 1024
NFFT = 2 * L               # 16384 = 128 * 128
NC = 8                     # cores
CPC = D // NC              # channels per core = 128
GC = 2                     # channels per group (DVE batching)
NG = CPC // GC             # 64 groups
U = GC * 2                 # signal slots per group (2 complex per channel)

# tuning knobs (build-time)
PW_ON_GPSIMD = os.environ.get("LC_PW_GPSIMD", "1") == "1"
FOLD_T1 = os.environ.get("LC_FOLD_T1", "0") == "1"   # fold T1 combine into F2
FOLD_T2 = os.environ.get("LC_FOLD_T2", "1") == "1"   # fold T2 combine into I2
DEBUG_DUMP = os.environ.get("LC_DEBUG", "0") == "1"  # dump group-0 intermediates


def _consts():
    j = np.arange(128)
    ang128 = 2 * np.pi * np.outer(j, j) / 128
    angN = 2 * np.pi * np.outer(j, j) / NFFT
    C, S = np.cos(ang128), np.sin(ang128)
    TC, TS = np.cos(angN), np.sin(angN)
    C64, S64 = C[:64], S[:64]

    arrs = {}
    # F1 rhs: rows 0:64 (zre): [cos | -sin]; rows 64:128 (zim): [sin | cos]
    arrs["f1k"] = np.block([[C64, -S64], [S64, C64]])
    # T1 quad consts: q = [Are|Aim|Are|Aim] * [TC|-TS|-TS|-TC]
    #   Bre = q0 - q1 ; Bim = q2 - q3
    arrs["t1q"] = np.concatenate([TC, -TS, -TS, -TC], axis=1)
    # T2 quad consts: q = [Qre|Qim|Qre|Qim] * [TC|TS|TS|-TC]
    #   C're = q0 - q1 ; C'im = q2 - q3
    arrs["t2q"] = np.concatenate([TC, TS, TS, -TC], axis=1)
    # F2 stationaries
    arrs["f2c"] = C
    arrs["f2s"] = S
    arrs["f2sn"] = -S
    arrs["f2cn"] = -C
    # I1 rhs: [-S | C | S]; rhs1 = [C|S] (cols 128:384), rhs2 = [-S|C] (0:256)
    arrs["i1m"] = np.concatenate([-S, C, S], axis=1)
    # I2 stationaries [k1, n1], n1 < 64
    arrs["gc"] = C[:, :64]
    arrs["gcn"] = -C[:, :64]
    arrs["gs"] = S[:, :64]
    arrs["gsn"] = -S[:, :64]
    return arrs


def _build_program():
    import concourse.bacc as bacc
    import concourse.mybir as mybir
    from concourse import tile

    f16 = mybir.dt.float16
    f32 = mybir.dt.float32

    nc = bacc.Bacc(None, target_bir_lowering=False, debug=False)

    # --- DRAM I/O ---
    xw = nc.dram_tensor("xw", (CPC, B, L), f16, kind="ExternalInput")
    kfq = nc.dram_tensor("kfq", (NG, 128, U, 384), f16, kind="ExternalInput")
    f1k_d = nc.dram_tensor("f1k", (128, 256), f16, kind="ExternalInput")
    t1u_d = nc.dram_tensor("t1u", (128, U, 512), f16, kind="ExternalInput")
    t2u_d = nc.dram_tensor("t2u", (128, U, 512), f16, kind="ExternalInput")
    f2c_d = nc.dram_tensor("f2c", (128, 128), f16, kind="ExternalInput")
    f2s_d = nc.dram_tensor("f2s", (128, 128), f16, kind="ExternalInput")
    f2sn_d = nc.dram_tensor("f2sn", (128, 128), f16, kind="ExternalInput")
    i1m_d = nc.dram_tensor("i1m", (128, 384), f16, kind="ExternalInput")
    gc_d = nc.dram_tensor("gc", (128, 64), f16, kind="ExternalInput")
    gcn_d = nc.dram_tensor("gcn", (128, 64), f16, kind="ExternalInput")
    gs_d = nc.dram_tensor("gs", (128, 64), f16, kind="ExternalInput")
    gsn_d = nc.dram_tensor("gsn", (128, 64), f16, kind="ExternalInput")
    yw = nc.dram_tensor("yw", (CPC, B, L), f16, kind="ExternalOutput")
    dbg = {}
    if DEBUG_DUMP:
        for nm, shp in (
            ("dA", (128, U, 256)), ("dB", (128, U, 256)), ("dR", (128, U, 256)),
            ("dP", (128, U, 256)), ("dQ", (128, U, 256)), ("dW", (128, U, 512)),
        ):
            dbg[nm] = nc.dram_tensor(nm, shp, f16, kind="ExternalOutput")

    with tile.TileContext(nc) as tc:
        with (
            tc.tile_pool(name="const", bufs=1) as constp,
            tc.tile_pool(name="kf", bufs=3) as kfp,
            tc.tile_pool(name="m", bufs=3) as mp,
            tc.tile_pool(name="work", bufs=3) as wp,
            tc.tile_pool(name="out", bufs=3) as op,
            tc.tile_pool(name="psum", bufs=1, space="PSUM") as pp,
        ):
            # constants, DMA'd once
            f1k = constp.tile([128, 256], f16)
            t1u = constp.tile([128, U, 512], f16)
            t2u = constp.tile([128, U, 512], f16)
            f2c = constp.tile([128, 128], f16)
            f2s = constp.tile([128, 128], f16)
            f2sn = constp.tile([128, 128], f16)
            i1m = constp.tile([128, 384], f16)
            gc = constp.tile([128, 64], f16)
            gcn = constp.tile([128, 64], f16)
            gs = constp.tile([128, 64], f16)
            gsn = constp.tile([128, 64], f16)
            for t, d in (
                (f1k, f1k_d), (t1u, t1u_d), (t2u, t2u_d), (f2c, f2c_d),
                (f2s, f2s_d), (f2sn, f2sn_d), (i1m, i1m_d),
                (gc, gc_d), (gcn, gcn_d), (gs, gs_d), (gsn, gsn_d),
            ):
                nc.sync.dma_start(t[:], d[:])

            for g in range(NG):
                kf = kfp.tile([128, U, 384], f16, tag="kf")
                nc.sync.dma_start(kf[:], kfq[g][:])

                m = mp.tile([128, U, 128], f16, tag="m")
                for j in range(GC):
                    c = g * GC + j
                    nc.sync.dma_start(
                        m[:, 2 * j : 2 * j + 2, :],
                        xw[c].rearrange(
                            "(s b) (a n) -> (b a) s n", s=2, n=128
                        ),
                    )

                # ---- F1: A[n2, k1] = m^T @ f1k per signal slot ----
                # pa: one 2-bank tile; one start/stop per bank.
                pa = pp.tile([128, U, 256], f32, tag="pa")
                for u in range(U):
                    nc.tensor.matmul(
                        pa[:, u, :], m[:, u, :], f1k[:],
                        start=(u % 2 == 0), stop=(u % 2 == 1),
                    )
                A_sb = wp.tile([128, U, 256], f16, tag="A")
                nc.scalar.copy(out=A_sb[:], in_=pa[:])
                if DEBUG_DUMP and g == 0:
                    nc.sync.dma_start(dbg["dA"][:], A_sb[:])

                # ---- T1 quads: tq = [A|A] * [TC|-TS|-TS|-TC] ----
                tq = wp.tile([128, U, 512], f16, tag="tq")
                nc.vector.tensor_mul(tq[:, :, 0:256], A_sb[:], t1u[:, :, 0:256])
                nc.vector.tensor_mul(tq[:, :, 256:512], A_sb[:], t1u[:, :, 256:512])

                # ---- T1 combines: Bre = q0 - q1, Bim = q2 - q3 ----
                B_sb = wp.tile([128, U, 256], f16, tag="B")
                nc.vector.tensor_sub(
                    B_sb[:, :, 0:128], tq[:, :, 0:128], tq[:, :, 128:256]
                )
                nc.vector.tensor_sub(
                    B_sb[:, :, 128:256], tq[:, :, 256:384], tq[:, :, 384:512]
                )
                if DEBUG_DUMP and g == 0:
                    nc.sync.dma_start(dbg["dB"][:], B_sb[:])

                # ---- F2 (fused N=512): re = C@Bre + S@Bim ; im = C@Bim - S@Bre
                pr_re = pp.tile([128, U, 128], f32, tag="prre")
                pr_im = pp.tile([128, U, 128], f32, tag="prim")
                ball_re = B_sb[:, :, 0:128]
                ball_im = B_sb[:, :, 128:256]
                nc.tensor.matmul(pr_re[:], f2c[:], ball_re, start=True, stop=False)
                nc.tensor.matmul(pr_im[:], f2c[:], ball_im, start=True, stop=False)
                nc.tensor.matmul(pr_re[:], f2s[:], ball_im, start=False, stop=True)
                nc.tensor.matmul(pr_im[:], f2sn[:], ball_re, start=False, stop=True)
                R_sb = wp.tile([128, U, 256], f16, tag="R")
                nc.scalar.copy(out=R_sb[:, :, 0:128], in_=pr_re[:])
                nc.scalar.copy(out=R_sb[:, :, 128:256], in_=pr_im[:])
                if DEBUG_DUMP and g == 0:
                    nc.sync.dma_start(dbg["dR"][:], R_sb[:])

                # ---- PW quads: pq = [R|R] * [Kre|Kim | Kim|Kre] ----
                pq = wp.tile([128, U, 512], f16, tag="pq")
                nc.vector.tensor_mul(pq[:, :, 0:256], R_sb[:], kf[:, :, 0:256])
                nc.vector.tensor_mul(pq[:, :, 256:512], R_sb[:], kf[:, :, 128:384])

                # ---- PW combines: Pre = q0 - q1 ; Pim = q2 + q3 ----
                P_sb = wp.tile([128, U, 256], f16, tag="P")
                eng = nc.gpsimd if PW_ON_GPSIMD else nc.vector
                eng.tensor_sub(
                    P_sb[:, :, 0:128], pq[:, :, 0:128], pq[:, :, 128:256]
                )
                eng.tensor_add(
                    P_sb[:, :, 128:256], pq[:, :, 256:384], pq[:, :, 384:512]
                )
                if DEBUG_DUMP and g == 0:
                    nc.sync.dma_start(dbg["dP"][:], P_sb[:])

                # ---- I1: Q[k1, n2] = P^T @ (C + iS), data-stationary ----
                # pc: one 2-bank tile; one start per bank (first mm of u=0/u=2),
                # one stop per bank (last mm of u=1/u=3).
                pc = pp.tile([128, U, 256], f32, tag="pc")
                for u in range(U):
                    nc.tensor.matmul(
                        pc[:, u, :], P_sb[:, u, 0:128], i1m[:, 128:384],
                        start=(u % 2 == 0), stop=False,
                    )
                    nc.tensor.matmul(
                        pc[:, u, :], P_sb[:, u, 128:256], i1m[:, 0:256],
                        start=False, stop=(u % 2 == 1),
                    )
                Q_sb = wp.tile([128, U, 256], f16, tag="Q")
                nc.scalar.copy(out=Q_sb[:], in_=pc[:])
                if DEBUG_DUMP and g == 0:
                    nc.sync.dma_start(dbg["dQ"][:], Q_sb[:])

                # ---- T2 quads: wq = [Q|Q] * [TC|TS|TS|-TC] ----
                wq = wp.tile([128, U, 512], f16, tag="wq")
                nc.vector.tensor_mul(wq[:, :, 0:256], Q_sb[:], t2u[:, :, 0:256])
                nc.vector.tensor_mul(wq[:, :, 256:512], Q_sb[:], t2u[:, :, 256:512])
                if DEBUG_DUMP and g == 0:
                    nc.sync.dma_start(dbg["dW"][:], wq[:])

                # ---- I2 (fused N=512) ----
                py_re = pp.tile([64, U, 128], f32, tag="pyre")
                py_im = pp.tile([64, U, 128], f32, tag="pyim")
                if FOLD_T2:
                    # C're = w0 - w1, C'im = w2 - w3 (wq slots)
                    # re: gc@w0 + gcn@w1 + gsn@w2 + gs@w3
                    # im: gc@w2 + gcn@w3 + gsn@w1 + gs@w0
                    def wv(k):
                        return wq[:, :, 128 * k : 128 * (k + 1)]

                    nc.tensor.matmul(py_re[:], gc[:], wv(0), start=True, stop=False)
                    nc.tensor.matmul(py_im[:], gc[:], wv(2), start=True, stop=False)
                    nc.tensor.matmul(py_re[:], gcn[:], wv(1), start=False, stop=False)
                    nc.tensor.matmul(py_im[:], gcn[:], wv(3), start=False, stop=False)
                    nc.tensor.matmul(py_re[:], gsn[:], wv(2), start=False, stop=False)
                    nc.tensor.matmul(py_im[:], gsn[:], wv(1), start=False, stop=False)
                    nc.tensor.matmul(py_re[:], gs[:], wv(3), start=False, stop=True)
                    nc.tensor.matmul(py_im[:], gs[:], wv(0), start=False, stop=True)
                else:
                    C_sb = wp.tile([128, U, 256], f16, tag="C")
                    nc.vector.tensor_sub(
                        C_sb[:, :, 0:128], wq[:, :, 0:128], wq[:, :, 128:256]
                    )
                    nc.vector.tensor_sub(
                        C_sb[:, :, 128:256], wq[:, :, 256:384], wq[:, :, 384:512]
                    )
                    cre = C_sb[:, :, 0:128]
                    cim = C_sb[:, :, 128:256]
                    nc.tensor.matmul(py_re[:], gc[:], cre, start=True, stop=False)
                    nc.tensor.matmul(py_im[:], gc[:], cim, start=True, stop=False)
                    nc.tensor.matmul(py_re[:], gsn[:], cim, start=False, stop=True)
                    nc.tensor.matmul(py_im[:], gs[:], cre, start=False, stop=True)

                ysb = op.tile([64, U, 256], f16, tag="ysb")
                nc.scalar.copy(out=ysb[:, :, 0:128], in_=py_re[:])
                nc.scalar.copy(out=ysb[:, :, 128:256], in_=py_im[:])

                # ---- store: one DMA per group ----
                # dst index (a, (c s), (b n)) -> yw[2g+c][2s+b][a*128+n]
                nc.sync.dma_start(
                    yw[2 * g : 2 * g + 2].rearrange(
                        "c (s b) (a n) -> a c s b n", s=2, n=128
                    ),
                    ysb[:].rearrange("p (c s) (b n) -> p c s b n", c=GC, n=128),
                )

    nc.compile()
    return nc


def _host_arrays():
    cst = _consts()
    arrs = {k: v.astype(np.float16) for k, v in cst.items()}
    # replicated twiddle quads (avoids 0-stride broadcast operands on DVE)
    arrs["t1u"] = np.ascontiguousarray(
        np.broadcast_to(arrs.pop("t1q")[:, None, :], (128, U, 512))
    )
    arrs["t2u"] = np.ascontiguousarray(
        np.broadcast_to(arrs.pop("t2q")[:, None, :], (128, U, 512))
    )
    arrs.pop("f2cn", None)
    return arrs


def _prep_inputs(x, filt):
    consts = _host_arrays()

    # filter spectrum: FFT of zero-padded filter; reshape(128,128) IS the
    # scrambled [k2, k1] layout of the on-device four-step forward.
    kpad = np.zeros((D, NFFT), np.float64)
    kpad[:, :L] = filt
    Kf = (np.fft.fft(kpad, axis=1) / NFFT).reshape(D, 128, 128)
    Kre = Kf.real.astype(np.float16)
    Kim = Kf.imag.astype(np.float16)
    # per-channel [128, 384] = [Kre | Kim | Kre]
    kq = np.concatenate([Kre, Kim, Kre], axis=2)  # (D, 128, 384)

    x16 = x.astype(np.float16)
    in_maps = []
    for ci in range(NC):
        sl = slice(ci * CPC, (ci + 1) * CPC)
        m = dict(consts)
        m["xw"] = np.ascontiguousarray(x16[:, :, sl].transpose(2, 0, 1))
        # kfq[g, p, u, :] = kq[core_base + 2g + u//2, p, :]
        m["kfq"] = np.ascontiguousarray(
            np.repeat(
                kq[sl].reshape(NG, GC, 128, 384).transpose(0, 2, 1, 3), 2, axis=2
            )
        )
        in_maps.append(m)
    return in_maps


def kernel(x: np.ndarray, filt: np.ndarray) -> np.ndarray:
    from concourse.bass_utils import run_bass_kernel_spmd

    assert x.shape == (B, L, D) and filt.shape == (D, L)
    x = np.ascontiguousarray(x, dtype=np.float32)
    filt = np.ascontiguousarray(filt, dtype=np.float32)

    in_maps = _prep_inputs(x, filt)
    nc = _build_program()
    res = run_bass_kernel_spmd(nc, in_maps, core_ids=list(range(NC)))

    y = np.empty((B, L, D), np.float32)
    for ci in range(NC):
        sl = slice(ci * CPC, (ci + 1) * CPC)
        y[:, :, sl] = res.results[ci]["yw"].astype(np.float32).transpose(1, 2, 0)
    return y


def run_profiled(inputs):
    """Build + run with NTFF tracing; returns BassKernelResults (test-only)."""
    from concourse.bass_utils import run_bass_kernel_spmd

    x = np.ascontiguousarray(inputs["x"], dtype=np.float32)
    filt = np.ascontiguousarray(inputs["filt"], dtype=np.float32)
    in_maps = _prep_inputs(x, filt)
    nc = _build_program()
    return run_bass_kernel_spmd(
        nc, in_maps, core_ids=list(range(NC)), trace=True
    )


if __name__ == "__main__":
    rng = np.random.default_rng(0)
    x = rng.standard_normal((B, L, D)).astype(np.float32)
    filt = rng.standard_normal((D, L)).astype(np.float32)
    y = kernel(x, filt)
    print("y", y.shape, y.dtype, float(np.abs(y).max()))
